# revision 3
# baseline (speedup 1.0000x reference)
"""Causal multi-head attention with RoPE on 8 Trainium2 NeuronCores, v2.

Sharding: core = (batch b, head-group hg): b = core//4, hg = core%4.
Each core computes 4 heads of one batch element end-to-end; the host sums
the 4 per-head-group output-projection partials per batch.

v2 vs baseline: one globally software-pipelined PE stream.  The per-chunk
phases (QKV projection+RoPE, causal attention, normalize, O-projection) are
interleaved so the PE always has independent work queued between dependent
score->exp->AV steps:

  phase qc: attention(qc) with filler = proj(qc+1) matmul chains,
            norm(qc-1) (no PE work), oproj sched {2:[0], 3:[1,2]}, inline
            per-head norm(3) during phase 3, oproj(3) in the tail.

Other changes vs baseline:
  - q/k/oT/wo in bf16 (scores/AV/O-proj matmuls bf16; ~4e-3 rel err)
  - RoPE pair-rotation via DVE stream_shuffle + sign-folded sin table
    (no PE permutation matmuls); muls split across DVE/GpSimd
  - softmax denominators: transposed-recip per chunk (per-head in phase 3),
    broadcast across partitions by a stride-0-source DMA (no PE broadcast
    matmuls, no PSUM bank for them)
  - diagonal causal trimming: score/EXP/AV skip the fully-masked leading
    q-columns of diagonal k-tiles; mask-mul only on the [128,128] triangle
  - AV emitted one group behind scores so PE never head-of-line blocks on
    Scalar's EXP.
"""

from collections import deque

import numpy as np

_B, _L, _D, _H, _HD = 2, 2048, 1024, 16, 64
_HPG = 4              # heads per group (per core)
_EG = _HPG * _HD      # 256
_NCORES = 8
_THETA = 10000.0
_QC = 512             # q-chunk width
_NQC = _L // _QC      # 4
_GK = 2               # k-tiles (128) per exp group
_NKC = _D // 128      # 8 contraction chunks for projections
_LC = 512             # l-chunk
_NLC = _L // _LC

_CACHE = {}


def _tf32(a):
    """Round float32 array to TF32 (fp32r): RNE to 10-bit mantissa."""
    b = np.ascontiguousarray(a, dtype=np.float32).view(np.uint32)
    b = (b + np.uint32(0xFFF) + ((b >> np.uint32(13)) & np.uint32(1))) \
        & np.uint32(0xFFFFE000)
    return b.view(np.float32)


class _Filler:
    """Deque of emission quanta, spread evenly over attention slots."""

    def __init__(self):
        self.q = deque()
        self.slots = 1

    def add(self, fn):
        self.q.append(fn)

    def emit(self):
        n = len(self.q) // max(self.slots, 1)
        for _ in range(n):
            if self.q:
                self.q.popleft()()
        self.slots = max(self.slots - 1, 1)

    def flush(self):
        while self.q:
            self.q.popleft()()


def _build_nc():
    from contextlib import ExitStack

    import concourse.mybir as mybir
    import concourse.tile as tile
    from concourse import bacc

    f32 = mybir.dt.float32
    f32r = mybir.dt.float32r
    bf16 = mybir.dt.bfloat16
    EXP = mybir.ActivationFunctionType.Exp

    nc = bacc.Bacc("TRN2", target_bir_lowering=False, debug=False,
                   enable_asserts=False)
    xT = nc.dram_tensor("xT", [_D, _L], bf16, kind="ExternalInput")
    wq = nc.dram_tensor("wq", [128, _NKC, _EG], bf16, kind="ExternalInput")
    wk = nc.dram_tensor("wk", [128, _NKC, _EG], bf16, kind="ExternalInput")
    wv = nc.dram_tensor("wv", [128, _NKC, _EG], bf16, kind="ExternalInput")
    wo = nc.dram_tensor("wo", [128, 2, _D], bf16, kind="ExternalInput")
    cs = nc.dram_tensor("cs", [128, _L], bf16, kind="ExternalInput")
    sn = nc.dram_tensor("sn", [128, _L], bf16, kind="ExternalInput")
    msk = nc.dram_tensor("msk", [128, 128], bf16, kind="ExternalInput")
    vones = nc.dram_tensor("vones", [128, _HD], bf16, kind="ExternalInput")
    onesr = nc.dram_tensor("onesr", [1, _HD], f32r, kind="ExternalInput")
    y = nc.dram_tensor("y", [_L, _D], f32, kind="ExternalOutput")

    pairmask = []
    for i in range(16):
        pairmask += [2 * i + 1, 2 * i]

    with tile.TileContext(nc) as tc, ExitStack() as ctx:
        persist = ctx.enter_context(tc.tile_pool(name="persist", bufs=1))
        qT_sb = persist.tile([128, 2, _L], bf16)
        kT_un = persist.tile([128, _HPG, _L], bf16)
        v_sb = persist.tile([128, _L // 128, _HPG, _HD + 4], bf16)
        oT_sb = persist.tile([128, 2, _L], bf16)
        wo_sb = persist.tile([128, 2, _D], bf16)
        msk_sb = persist.tile([128, 128], bf16)
        wq_sb = persist.tile([128, _NKC, _EG], bf16)
        wk_sb = persist.tile([128, _NKC, _EG], bf16)
        wv_sb = persist.tile([128, _NKC, _EG], bf16)
        cs_sb = persist.tile([128, _L], bf16)
        sn_sb = persist.tile([128, _L], bf16)
        ones_sb = persist.tile([1, 64], f32r)

        xtp = ctx.enter_context(tc.tile_pool(name="xtp", bufs=2))
        rtp = ctx.enter_context(tc.tile_pool(name="rtp", bufs=3))
        ptp = ctx.enter_context(tc.tile_pool(name="ptp", bufs=6))
        ocp = ctx.enter_context(tc.tile_pool(name="ocp", bufs=3))
        nrm = ctx.enter_context(tc.tile_pool(name="nrm", bufs=2))
        # PSUM budget (8 banks): sps 2x2 + ops 1x1 + pjp 1x1 + scr 2x1
        sps = ctx.enter_context(tc.tile_pool(name="sps", bufs=2, space="PSUM"))
        ops = ctx.enter_context(tc.tile_pool(name="ops", bufs=1, space="PSUM"))
        pjp = ctx.enter_context(tc.tile_pool(name="pjp", bufs=1, space="PSUM"))
        scr = ctx.enter_context(tc.tile_pool(name="scr", bufs=2, space="PSUM"))

        # --- input loads ---
        xT_r = xT.rearrange("(c p) l -> p c l", p=128)
        xts = {}
        nc.vector.memset(kT_un, 0.0)

        def load_xt(lc):
            xt = xtp.tile([128, _NKC, _LC], bf16, tag="xt", name=f"xt{lc}")
            for kc in range(_NKC):
                nc.sync.dma_start(
                    out=xt[:, kc, :],
                    in_=xT_r[:, kc, lc * _LC:(lc + 1) * _LC])
            xts[lc] = xt

        # interleave per-kc so the first projection chain can start as soon
        # as the first slices land, not after the full weight load
        xt0 = xtp.tile([128, _NKC, _LC], bf16, tag="xt", name="xt0")
        xts[0] = xt0
        for half in range(2):
            ks = slice(half * 4, half * 4 + 4)
            nc.sync.dma_start(out=wk_sb[:, ks, :], in_=wk[:, ks, :])
            for kc in range(half * 4, half * 4 + 4):
                nc.sync.dma_start(out=xt0[:, kc, :],
                                  in_=xT_r[:, kc, 0:_LC])
        for half in range(2):
            ks = slice(half * 4, half * 4 + 4)
            nc.sync.dma_start(out=wq_sb[:, ks, :], in_=wq[:, ks, :])
        for half in range(2):
            ks = slice(half * 4, half * 4 + 4)
            nc.sync.dma_start(out=wv_sb[:, ks, :], in_=wv[:, ks, :])
        load_xt(1)
        nc.gpsimd.dma_start(out=cs_sb, in_=cs[:, :])
        nc.gpsimd.dma_start(out=sn_sb, in_=sn[:, :])
        nc.gpsimd.dma_start(out=msk_sb, in_=msk[:, :])
        nc.gpsimd.dma_start(out=ones_sb, in_=onesr[:, :])
        nc.gpsimd.dma_start(out=wo_sb, in_=wo[:, :, :])
        nc.gpsimd.dma_start(
            out=v_sb[:, :, :, _HD:_HD + 1],
            in_=vones.rearrange("p (a b) -> p a b",
                                a=_L // 128).unsqueeze(3))

        def proj_ps(tag):
            pool = pjp if tag == "ps" else scr
            return pool.tile([128, _LC], f32, tag=tag, name=f"pj_{tag}")

        # ---------- filler quanta builders ----------
        def add_qk_chain(fil, lc, w_sb, dst, c, tag):
            ls = slice(lc * _LC, (lc + 1) * _LC)
            st = {}

            def mm(kc):
                def f():
                    if kc == 0:
                        st["ps"] = proj_ps(tag)
                    nc.tensor.matmul(
                        st["ps"], w_sb[:, kc, c * 128:(c + 1) * 128],
                        xts[lc][:, kc, :],
                        start=(kc == 0), stop=(kc == _NKC - 1),
                        skip_group_check=True)
                return f

            for kc in range(_NKC):
                fil.add(mm(kc))

            def rope():
                # out = qp*cos + pairswap(qp)*sn'  (sn' sign-folded)
                qp = rtp.tile([128, _LC], bf16, tag="qp", name="qp")
                nc.vector.tensor_copy(qp, st["ps"])
                shf = rtp.tile([128, _LC], bf16, tag="shf", name="shf")
                nc.vector.stream_shuffle(shf, qp, mask=pairmask)
                if dst is qT_sb:
                    out = dst[:, c, ls]
                else:
                    out = rtp.tile([128, _LC], bf16, tag="ktp", name="ktp")
                nc.vector.tensor_mul(out, qp, cs_sb[:, ls])
                tmp = rtp.tile([128, _LC], bf16, tag="rtmp", name="rtmp")
                nc.vector.tensor_mul(tmp, shf, sn_sb[:, ls])
                nc.gpsimd.tensor_add(out, out, tmp)
                if dst is not qT_sb:
                    # unpack the two heads into zero-padded per-head planes
                    nc.gpsimd.dma_start(out=kT_un[0:64, 2 * c, ls],
                                        in_=out[0:64, :])
                    nc.gpsimd.dma_start(out=kT_un[64:128, 2 * c + 1, ls],
                                        in_=out[64:128, :])

            fil.add(rope)

        def add_v_chain(fil, lc, j, tag):
            lt = lc * (_LC // 128) + j
            st = {}

            def mm(kc):
                def f():
                    if kc == 0:
                        st["ps"] = proj_ps(tag)
                    nc.tensor.matmul(
                        st["ps"][:, :_EG],
                        xts[lc][:, kc, j * 128:(j + 1) * 128],
                        wv_sb[:, kc, :],
                        start=(kc == 0), stop=(kc == _NKC - 1),
                        skip_group_check=True)
                return f

            for kc in range(_NKC):
                fil.add(mm(kc))

            def cp():
                nc.scalar.copy(
                    v_sb[:, lt, :, :_HD],
                    st["ps"][:, :_EG].rearrange("p (h e) -> p h e", h=_HPG))

            fil.add(cp)

        def add_proj(fil, lc, alt, spacers=()):
            spacers = deque(spacers)
            idx = 0
            work = [("qk", wk_sb, kT_un, 0), ("qk", wq_sb, qT_sb, 0),
                    ("v", None, None, 0), ("v", None, None, 1),
                    ("v", None, None, 2), ("v", None, None, 3),
                    ("qk", wk_sb, kT_un, 1), ("qk", wq_sb, qT_sb, 1)]
            for kind, w_sb, dst, cj in work:
                tag = "op" if (alt and idx % 2) else "ps"
                if kind == "qk":
                    add_qk_chain(fil, lc, w_sb, dst, cj, tag)
                else:
                    add_v_chain(fil, lc, cj, tag)
                if spacers:
                    fil.add(spacers.popleft())
                idx += 1
            while spacers:
                fil.add(spacers.popleft())

        # den_rows[(qc, h)] = drow tile [1, 512] f32r (reciprocal row)
        den_rows = {}

        def add_den_batch(qc, otcs, hs):
            """Transposed den gather + recip + per-head row restore."""
            nh = len(hs)
            dsb = nrm.tile([128, _HPG * 4], f32, tag="dsb", name="dsb")
            for j, h in enumerate(hs):
                nc.gpsimd.dma_start(
                    out=dsb[:, j * 4:j * 4 + 4],
                    in_=otcs[h][_HD:_HD + 1, :])
            drec = nrm.tile([128, _HPG * 4], f32, tag="drec", name="drec")
            nc.vector.reciprocal(drec[:, :nh * 4], dsb[:, :nh * 4])
            drecr = nrm.tile([128, _HPG * 4], f32r, tag="drecr", name="drecr")
            nc.scalar.copy(drecr[:, :nh * 4], drec[:, :nh * 4])
            for j, h in enumerate(hs):
                drow = nrm.tile([1, _QC], f32r, tag="drow", bufs=5,
                                name=f"drow{qc}_{h}")
                nc.gpsimd.dma_start(out=drow,
                                    in_=drecr[:, j * 4:j * 4 + 4])
                den_rows[(qc, h)] = drow

        def norm_head(qc_, h, otc):
            c, pb = h // 2, 64 * (h % 2)
            qs = slice(qc_ * _QC, (qc_ + 1) * _QC)
            bc = scr.tile([128, 512], f32, tag="op", name="bc")
            nc.tensor.matmul(bc[0:64, :], ones_sb, den_rows.pop((qc_, h)),
                             start=True, stop=True, skip_group_check=True)
            otn = ocp.tile([64, _QC], bf16, tag="otn", bufs=2, name="otn")
            nc.vector.tensor_mul(otn, otc[0:_HD, :], bc[0:64, :])
            nc.gpsimd.dma_start(out=oT_sb[pb:pb + 64, c, qs], in_=otn)

        def oproj_tile(qc_, j, ob_eng, bank="scr"):
            lt = qc_ * (_QC // 128) + j
            ob = ocp.tile([128, _D], f32, tag="ob", name="ob")
            for n in range(2):
                if bank == "scr":
                    op = scr.tile([128, 512], f32, tag="op", name="op")
                else:
                    op = ops.tile([128, _QC], f32, tag="ot", name="opt")
                for cc in range(2):
                    nc.tensor.matmul(
                        op, oT_sb[:, cc, lt * 128:(lt + 1) * 128],
                        wo_sb[:, cc, n * 512:(n + 1) * 512],
                        start=(cc == 0), stop=(cc == 1),
                        skip_group_check=True)
                if ob_eng == "v":
                    nc.vector.tensor_copy(ob[:, n * 512:(n + 1) * 512], op)
                else:
                    nc.scalar.copy(ob[:, n * 512:(n + 1) * 512], op)
            nc.sync.dma_start(
                out=y[lt * 128:(lt + 1) * 128, :], in_=ob)

        # ---------- attention ----------
        def scores(st, g):
            qc, c, pb = st["qc"], st["c"], st["pb"]
            q0 = qc * _QC
            sp = sps.tile([128, _GK * _QC], f32, tag="sp", name="sp")
            pt = ptp.tile([128, _GK * _QC], bf16, tag="pt", name="pt")
            diag = []
            for i in range(_GK):
                kt = _GK * g + i
                dj = kt - qc * (_QC // 128)
                o = dj * 128 if dj > 0 else 0
                nc.tensor.matmul(
                    sp[:, i * _QC + o:(i + 1) * _QC],
                    kT_un[:, st["h"], kt * 128:(kt + 1) * 128],
                    qT_sb[:, c, q0 + o:q0 + _QC],
                    start=True, stop=True, skip_group_check=True)
                if dj >= 0:
                    diag.append((i, dj))
            if diag:
                for i in range(_GK):
                    dj = _GK * g + i - qc * (_QC // 128)
                    o = dj * 128 if dj > 0 else 0
                    nc.scalar.activation(
                        pt[:, i * _QC + o:(i + 1) * _QC],
                        sp[:, i * _QC + o:(i + 1) * _QC], EXP, scale=0.125)
                for i, dj in diag:
                    o = i * _QC + dj * 128
                    nc.vector.tensor_mul(pt[:, o:o + 128], pt[:, o:o + 128],
                                         msk_sb)
            else:
                nc.scalar.activation(pt, sp, EXP, scale=0.125)
            st["pt"][g] = pt

        def av(st, g):
            qc, h = st["qc"], st["h"]
            nkt = (qc + 1) * (_QC // 128)
            if "ot" not in st:
                st["ot"] = ops.tile([128, _QC], f32, tag="ot",
                                    name=f"ot{qc}_{h}")
            pt = st["pt"].pop(g)
            for i in range(_GK):
                kt = _GK * g + i
                dj = kt - qc * (_QC // 128)
                o = dj * 128 if dj > 0 else 0
                nc.tensor.matmul(
                    st["ot"][:_HD + 1, o:_QC], v_sb[:, kt, h, :_HD + 1],
                    pt[:, i * _QC + o:(i + 1) * _QC],
                    start=(kt == 0), stop=(kt == nkt - 1),
                    skip_group_check=True)

        def finish_head(st, fil):
            qc, h = st["qc"], st["h"]
            av(st, st["ngr"] - 1)
            otc = ocp.tile([_HD + 1, _QC], f32, tag="otc", bufs=6,
                           name=f"otc{qc}_{h}")
            nc.vector.tensor_copy(otc, st["ot"][:_HD + 1, :])
            st["otc"] = otc

        def attention_phase(qc, fil):
            last = qc == _NQC - 1
            ngr = (qc + 1) * (_QC // 128) // _GK
            fil.slots = _HPG * ngr
            sts = []
            pend = None
            for h in range(_HPG):
                st = {"qc": qc, "h": h, "c": h // 2, "pb": 64 * (h % 2),
                      "ngr": ngr, "pt": {}}
                sts.append(st)
                scores(st, 0)
                if pend is not None:
                    finish_head(pend, fil)
                    if last and pend["h"] == 1:
                        add_den_batch(qc, {0: sts[0]["otc"],
                                           1: sts[1]["otc"]}, [0, 1])
                        for hh in (0, 1):
                            fil.add(lambda hh=hh, o=sts[hh]["otc"]:
                                    norm_head(qc, hh, o))
                    elif last and pend["h"] == 2:
                        add_den_batch(qc, {2: sts[2]["otc"]}, [2])
                        fil.add(lambda o=sts[2]["otc"]:
                                norm_head(qc, 2, o))
                for g in range(1, ngr):
                    scores(st, g)
                    fil.emit()
                    av(st, g - 1)
                fil.emit()
                pend = st
            finish_head(pend, fil)
            fil.flush()
            return [s["otc"] for s in sts]

        # ---------- main schedule ----------
        # prologue: proj(0) dense, double-buffered through ps/op banks
        fil = _Filler()
        add_proj(fil, 0, alt=True)
        fil.flush()

        oproj_sched = {2: [(0, "s")], 3: [(1, "v"), (2, "v")]}
        prev_otcs = None
        for qc in range(_NQC):
            fil = _Filler()
            if qc + 2 < _NLC:
                fil.add(lambda lc=qc + 2: load_xt(lc))
            if qc >= 1:
                for h in range(_HPG):
                    fil.add(lambda h=h, o=prev_otcs[h], q=qc - 1:
                            norm_head(q, h, o))
            spacers = []
            for qc2, eng in oproj_sched.get(qc, []):
                for j in range(_QC // 128):
                    spacers.append(lambda q=qc2, j=j, e=eng:
                                   oproj_tile(q, j, e))
            if qc + 1 < _NLC:
                add_proj(fil, qc + 1, alt=True, spacers=spacers)
            else:
                for s in spacers:
                    fil.add(s)
            otcs = attention_phase(qc, fil)
            if qc < _NQC - 1:
                add_den_batch(qc, otcs, [0, 1, 2, 3])
            prev_otcs = otcs
        # tail: last norm + oproj of the last chunk; alternate psum banks
        # (sps pool is idle now) so consecutive matmuls overlap the copies
        add_den_batch(_NQC - 1, prev_otcs, [3])
        norm_head(_NQC - 1, 3, prev_otcs[3])
        for j in range(_QC // 128):
            lt = (_NQC - 1) * (_QC // 128) + j
            ob = ocp.tile([128, _D], f32, tag="ob", name="ob")
            for n in range(2):
                op = sps.tile([128, _GK * _QC], f32, tag="sp", name="opt")
                for cc in range(2):
                    nc.tensor.matmul(
                        op[:, :512], oT_sb[:, cc, lt * 128:(lt + 1) * 128],
                        wo_sb[:, cc, n * 512:(n + 1) * 512],
                        start=(cc == 0), stop=(cc == 1),
                        skip_group_check=True)
                if n == 0:
                    nc.vector.tensor_copy(ob[:, :512], op[:, :512])
                else:
                    nc.scalar.copy(ob[:, 512:], op[:, :512])
            nc.sync.dma_start(out=y[lt * 128:(lt + 1) * 128, :], in_=ob)
    nc.compile()
    return nc


def get_nc():
    if "nc" not in _CACHE:
        _CACHE["nc"] = _build_nc()
    return _CACHE["nc"]


def make_in_maps(x, token_positions, Q, K, V, O_w):
    """Host-side sharding: per-core input dict (core = b*4 + hg)."""
    import ml_dtypes
    bf16 = ml_dtypes.bfloat16
    x = np.asarray(x, dtype=np.float32)
    tp = np.asarray(token_positions)
    Q = np.asarray(Q, dtype=np.float32)
    K = np.asarray(K, dtype=np.float32)
    V = np.asarray(V, dtype=np.float32)
    O_w = np.asarray(O_w, dtype=np.float32)

    # RoPE tables, [128, L]: rows 0..63 head-local e (pairwise repeat),
    # rows 64..127 a copy (two heads share one partition tile).
    # sn has the pair-rotation sign folded in: row 2i = -sin_i, 2i+1 = +sin_i.
    i = np.arange(_HD // 2, dtype=np.float64)
    denom = _THETA ** (2.0 * i / _HD)
    ang = tp.astype(np.float64)[None, :] / denom[:, None]  # [32, L]
    cs64 = np.repeat(np.cos(ang), 2, axis=0)
    sn64 = np.repeat(np.sin(ang), 2, axis=0)
    sn64[0::2] *= -1.0
    cs = np.vstack([cs64, cs64]).astype(bf16)
    sn = np.vstack([sn64, sn64]).astype(bf16)

    # triangular mask for the leading 128-col block of diagonal k-tiles
    pp = np.arange(128)[:, None]
    ff = np.arange(128)[None, :]
    msk = (ff >= pp).astype(bf16)

    Qr = Q.reshape(_H, _HD, _D)
    Kr = K.reshape(_H, _HD, _D)
    Vr = V.reshape(_H, _HD, _D)

    in_maps = []
    xT = [np.ascontiguousarray(x[b].T).astype(bf16) for b in range(_B)]
    for core in range(_NCORES):
        b, hg = core // 4, core % 4
        hs = slice(hg * _HPG, (hg + 1) * _HPG)
        wo_t = O_w[:, hg * _EG:(hg + 1) * _EG].T   # [256, 1024]
        in_maps.append({
            "xT": xT[b],
            "wq": np.ascontiguousarray(
                Qr[hs].reshape(_EG, _D).T.reshape(_NKC, 128, _EG)
                .transpose(1, 0, 2)).astype(bf16),
            "wk": np.ascontiguousarray(
                Kr[hs].reshape(_EG, _D).T.reshape(_NKC, 128, _EG)
                .transpose(1, 0, 2)).astype(bf16),
            "wv": np.ascontiguousarray(
                Vr[hs].reshape(_EG, _D).T.reshape(_NKC, 128, _EG)
                .transpose(1, 0, 2)).astype(bf16),
            "wo": np.ascontiguousarray(
                wo_t.reshape(2, 128, _D).transpose(1, 0, 2)).astype(bf16),
            "cs": cs, "sn": sn, "msk": msk,
            "vones": np.ones((128, _HD), bf16),
            "onesr": np.ones((1, _HD), np.float32),
        })
    return in_maps


def run_on_hw(in_maps, trace=False, **kw):
    from concourse.bass_utils import run_bass_kernel_spmd
    nc = get_nc()
    return run_bass_kernel_spmd(nc, in_maps, core_ids=list(range(_NCORES)),
                                trace=trace, **kw)


def kernel(x, token_positions, Q, K, V, O_w):
    in_maps = make_in_maps(x, token_positions, Q, K, V, O_w)
    res = run_on_hw(in_maps)
    out = np.zeros((_B, _L, _D), dtype=np.float32)
    for core in range(_NCORES):
        out[core // 4] += res.results[core]["y"]
    return out
